# revision 45
# baseline (speedup 1.0000x reference)
"""Blocksparse conv2d (3x3, stride 1, pad 1) on 8 Trainium2 NeuronCores.

Strategy
--------
Data-parallel over batch: 16 images -> 2 per core, identical SPMD program.

The mask zeroes whole 32x32 (cout, cin) channel blocks; the host inspects
the runtime mask and specializes the schedule: only surviving input-channel
blocks are shipped/loaded (K_used channels).  When 2*K_used <= 128 the two
images of a core are PAIRED across PE row groups (img0 on partitions
0:K_used, img1 on K_used:2*K_used) so their matmul streams run concurrently
on independent row tiles of the systolic array -- full-array throughput
without duplicating any x data (the previous replication scheme doubled the
x HBM traffic for the same PE rate).

Conv is lowered to 9 shifted matmuls accumulating in PSUM.  The host
pre-pads each image with a zero border (130x130), so every tap is a clean
2D-strided view of one flat SBUF buffer -- no edge-column fixup matmuls at
all.  The x load is chunked into 12 row-band HWDGE DMAs so the matmul
pipeline starts as soon as the first small band lands instead of waiting
~25us for the full load.

x and the host-side masked/transposed/replicated weights are converted to
bf16 on the host (error ~2e-3, well under the 2e-2 gate; f32r measured ~2
cycles/col at N=512 with the strided rhs view, bf16 streams at 1), halving
x HBM traffic and keeping every cast off the device's critical path.

Windows are processed one pair at a time (one PSUM bank per image), so the
8-bank pool holds 4 pairs in flight and bank-recycle waits resolve ~3 pairs
early: the measured steady state is the PE's 218ns/pair streaming limit
with LDWEIGHTS fully hidden by the pull-ahead window.  (Multi-window groups
measured ~0.9us of PSUM-recycle stall per group boundary.)  Bias is fused
into the PSUM->SBUF copy, alternating scalar/vector engines; y stores
alternate the two HWDGE rings, with tapered final chunks so the last store
drains in ~1.5us.
"""

import ml_dtypes
import numpy as np
from contextlib import ExitStack

import concourse.bass as bass
import concourse.tile as tile
from concourse import mybir, bacc
from concourse import bass_utils
# Problem shape (hardcoded per contract)
B, CIN, COUT, H, W = 16, 128, 128, 128, 128
KH, KW = 3, 3
BLK = 32
NCORES = 8
BPC = B // NCORES            # images per core
PH, PW = H + 2, W + 2        # host zero-padded image (130 x 130)
FLAT = PH * PW

RPW = 4                      # output rows per PSUM window (N = 512 = full bank)
NWIN = H // RPW              # 32 windows
CHUNKS = [8, 8, 8, 4, 2, 2]  # windows per output-DMA chunk (tapered tail so the
                             # final y store drains fast; finer 1-window tails
                             # measured slower -- extra stores cost more in
                             # end-of-kernel completion waits than they save)
# One window-pair per matmul group: steady state runs at the 218ns/pair
# streaming limit with LDWEIGHTS hidden by the PE's pull-ahead, and the
# 8-bank PSUM pool holds 4 pairs in flight, so a pair's bank-reuse wait is
# always satisfied ~3 pairs ahead of time.  Larger groups measured ~0.9us
# of PSUM-recycle stall at every group boundary.
GROUPS = [[1] * c for c in CHUNKS]

_cache = {}
_last_in_maps = None


def _build(n_ib, paired):
    """Build + compile the per-core SPMD program.

    n_ib:   number of surviving 32-channel input blocks (1..4)
    paired: both images share the partition dim on separate PE row groups
    """
    K_used = BLK * n_ib
    reps = 2 if paired else 1
    DK = reps * K_used
    assert DK <= 128

    nc = bacc.Bacc("TRN2", target_bir_lowering=False, debug=False)
    f32 = mybir.dt.float32
    bf16 = mybir.dt.bfloat16

    # x and wT arrive in DRAM already bf16 (host converts): halves the x HBM
    # read and removes every on-chip cast from the load critical path.
    x_in = nc.dram_tensor("x", [BPC, K_used, PH, PW], bf16, kind="ExternalInput").ap()
    w_in = nc.dram_tensor("wt", [DK, KH * KW, COUT], bf16, kind="ExternalInput").ap()
    b_in = nc.dram_tensor("bias", [COUT], f32, kind="ExternalInput").ap()
    y_out = nc.dram_tensor("y", [BPC, COUT, H, W], f32, kind="ExternalOutput").ap()

    # x-load chunk boundaries (padded-image rows).  Window w reads padded rows
    # 4w..4w+5, so a boundary at 4k+6 releases windows 0..k.  Fine-grained
    # early chunks let the matmul pipeline start early; coarser later ones
    # keep the DMA count low.  (Merging to 8 chunks / splitting the weight
    # load / moving chunk 0 to the scalar ring all measured ~5us slower.)
    bounds = [0, 6, 14, 22, 30]
    while bounds[-1] + 16 < PH:
        bounds.append(bounds[-1] + 16)
    bounds.append(PH)

    with tile.TileContext(nc) as tc:
        with ExitStack() as ctx:
            singles = ctx.enter_context(tc.tile_pool(name="singles", bufs=1))
            stage_pool = ctx.enter_context(tc.tile_pool(name="ystage", bufs=4))
            psum_pool = ctx.enter_context(
                tc.tile_pool(name="psum", bufs=8, space="PSUM")
            )

            # Start-path layout (measured: first matmul 13.3 -> 11.7us):
            # chunk 0 goes on the Activation HWDGE ring so it drains in
            # parallel with chunk 1 on the SP ring, and the tap-0 weight
            # slice loads separately so the first LDWEIGHTS gates on 33KB,
            # not the whole 295KB weight tensor.
            wT = singles.tile([DK, KH * KW, COUT], bf16, name="wT2")
            bias_sb = singles.tile([COUT, 1], f32, name="bias_sb")

            def load_chunks(xbt, src):
                lo, hi = bounds[0] * PW, bounds[1] * PW
                nc.scalar.dma_start(out=xbt[:, lo:hi], in_=src[:, lo:hi])
                nc.scalar.dma_start(out=wT[:, 0:1, :], in_=w_in[:, 0:1, :])
                nc.scalar.dma_start(out=wT[:, 1:, :], in_=w_in[:, 1:, :])
                nc.scalar.dma_start(out=bias_sb, in_=b_in.unsqueeze(1))
                for k in range(1, len(bounds) - 1):
                    lo, hi = bounds[k] * PW, bounds[k + 1] * PW
                    nc.sync.dma_start(out=xbt[:, lo:hi], in_=src[:, lo:hi])

            if paired:
                xb = singles.tile([DK, FLAT], bf16, name="xb")
                load_chunks(xb, x_in.rearrange("b c h w -> (b c) (h w)"))
                xbufs = [xb] * BPC
                img_base = [i * K_used for i in range(BPC)]
            else:
                xbufs, img_base = [], []
                for b in range(BPC):
                    xbi = singles.tile([K_used, FLAT], bf16, name=f"xb{b}")
                    load_chunks(xbi, x_in[b].rearrange("c h w -> c (h w)"))
                    xbufs.append(xbi)
                    img_base.append(0)

            assert sum(CHUNKS) == NWIN
            assert [sum(g) for g in GROUPS] == CHUNKS
            c0 = 0
            for nwc, chunk_groups in zip(CHUNKS, GROUPS):
                wins = list(range(c0, c0 + nwc))
                chunk_r0 = RPW * c0
                chunk_nr = RPW * len(wins)
                c0 += nwc
                stages = [
                    stage_pool.tile(
                        [COUT, RPW * max(CHUNKS), W], f32, tag="stage",
                        name=f"st{b}_{chunk_r0}",
                    )
                    for b in range(BPC)
                ]
                g0 = 0
                for gsz in chunk_groups:
                    group = wins[g0 : g0 + gsz]
                    g0 += gsz
                    ps = {}
                    for b in range(BPC):
                        for w in group:
                            ps[(b, w)] = psum_pool.tile(
                                [128, 512], f32, tag="ps", name=f"ps{b}_{w}"
                            )
                    # tap-outer, image-inner: the two images' row groups
                    # alternate so each group's LDWEIGHTS hides behind the
                    # other group's matmuls.
                    for t in range(KH * KW):
                        dh, dw = divmod(t, KW)
                        for b in range(BPC):
                            base = img_base[b]
                            xbi = xbufs[b]
                            lhsT = wT[base : base + K_used, t, :]
                            for w in group:
                                q0 = (RPW * w + dh) * PW + dw
                                v = xbi[base : base + K_used, q0 : q0 + 1]
                                rhs = bass.AP(
                                    tensor=v.tensor,
                                    offset=v.offset,
                                    ap=[list(v.ap[0]), [PW, RPW], [1, W]],
                                )
                                nc.tensor.matmul(
                                    ps[(b, w)][:, : RPW * W],
                                    lhsT,
                                    rhs,
                                    start=(t == 0),
                                    stop=(t == KH * KW - 1),
                                )
                    # copy-out with fused bias, split across ACT and DVE
                    for b in range(BPC):
                        for w in group:
                            r0 = RPW * w
                            ps_v = ps[(b, w)][:, : RPW * W].rearrange(
                                "p (r s) -> p r s", s=W
                            )
                            dst = stages[b][:, r0 - chunk_r0 : r0 - chunk_r0 + RPW, :]
                            if (w + b) % 2 == 0:
                                nc.scalar.activation(
                                    out=dst,
                                    in_=ps_v,
                                    func=mybir.ActivationFunctionType.Identity,
                                    bias=bias_sb,
                                    scale=1.0,
                                )
                            else:
                                nc.vector.tensor_scalar_add(
                                    out=dst, in0=ps_v, scalar1=bias_sb
                                )
                # alternate the two HWDGE rings (SP / Activation) so the small
                # final stores don't queue behind every earlier large store
                for b in range(BPC):
                    eng = nc.sync if b == 0 else nc.scalar
                    eng.dma_start(
                        out=y_out[b][:, chunk_r0 : chunk_r0 + chunk_nr, :],
                        in_=stages[b][:, :chunk_nr, :],
                    )

    nc.compile()
    return nc


def kernel(x, weight, bias, mask):
    x = np.ascontiguousarray(np.asarray(x, dtype=np.float32))
    weight = np.asarray(weight, dtype=np.float32)
    bias = np.ascontiguousarray(np.asarray(bias, dtype=np.float32))
    mask = np.asarray(mask, dtype=np.float32)

    # --- host-side schedule specialization from the runtime mask ----------
    wm = weight * mask
    blk_any = (
        np.abs(wm).reshape(COUT, CIN // BLK, BLK, KH, KW).sum(axis=(0, 2, 3, 4)) > 0
    )
    used_ibs = [ib for ib in range(CIN // BLK) if blk_any[ib]] or [0]
    n_ib = len(used_ibs)
    K_used = BLK * n_ib
    paired = (BPC == 2) and (2 * K_used <= 128)
    reps = 2 if paired else 1

    used_ch = np.concatenate(
        [np.arange(ib * BLK, (ib + 1) * BLK) for ib in used_ibs]
    )

    key = (n_ib, paired)
    if key not in _cache:
        _cache[key] = _build(n_ib, paired)
    nc = _cache[key]

    # wT[rep*K_used + c, t, o] = (w*m)[o, used_ch[c], tap t], host-cast bf16
    wT = wm[:, used_ch].reshape(COUT, K_used, KH * KW).transpose(1, 2, 0)
    wT = np.ascontiguousarray(
        np.concatenate([wT] * reps, axis=0).astype(ml_dtypes.bfloat16)
    )

    # zero-padded x (130x130) restricted to the used channels, host-cast bf16
    xp = np.zeros((B, K_used, PH, PW), dtype=ml_dtypes.bfloat16)
    xp[:, :, 1 : H + 1, 1 : W + 1] = x[:, used_ch].astype(ml_dtypes.bfloat16)

    in_maps = []
    for core in range(NCORES):
        xs = np.ascontiguousarray(xp[core * BPC : (core + 1) * BPC])
        in_maps.append({"x": xs, "wt": wT, "bias": bias})

    global _last_in_maps
    _last_in_maps = in_maps

    res = bass_utils.run_bass_kernel_spmd(nc, in_maps, core_ids=list(range(NCORES)))
    y = np.concatenate([res.results[c]["y"] for c in range(NCORES)], axis=0)
    return y


# revision 46
# speedup vs baseline: 1.0415x; 1.0415x over previous
"""Blocksparse conv2d (3x3, stride 1, pad 1) on 8 Trainium2 NeuronCores.

Strategy
--------
Data-parallel over batch: 16 images -> 2 per core, identical SPMD program.

The mask zeroes whole 32x32 (cout, cin) channel blocks; the host inspects
the runtime mask and specializes the schedule: only surviving input-channel
blocks are shipped/loaded (K_used channels).  When 2*K_used <= 128 the two
images of a core are PAIRED across PE row groups (img0 on partitions
0:K_used, img1 on K_used:2*K_used) so their matmul streams run concurrently
on independent row tiles of the systolic array -- full-array throughput
without duplicating any x data (the previous replication scheme doubled the
x HBM traffic for the same PE rate).

Conv is lowered to 9 shifted matmuls accumulating in PSUM.  The host
pre-pads each image with a zero border (130x130), so every tap is a clean
2D-strided view of one flat SBUF buffer -- no edge-column fixup matmuls at
all.  The x load is chunked into 12 row-band HWDGE DMAs so the matmul
pipeline starts as soon as the first small band lands instead of waiting
~25us for the full load.

x and the host-side masked/transposed/replicated weights are converted to
bf16 on the host (error ~2e-3, well under the 2e-2 gate; f32r measured ~2
cycles/col at N=512 with the strided rhs view, bf16 streams at 1), halving
x HBM traffic and keeping every cast off the device's critical path.

Windows are processed one pair at a time (one PSUM bank per image), so the
8-bank pool holds 4 pairs in flight and bank-recycle waits resolve ~3 pairs
early: the measured steady state is the PE's 218ns/pair streaming limit
with LDWEIGHTS fully hidden by the pull-ahead window.  (Multi-window groups
measured ~0.9us of PSUM-recycle stall per group boundary.)  Bias is fused
into the PSUM->SBUF copy, alternating scalar/vector engines; y stores
alternate the two HWDGE rings, with tapered final chunks so the last store
drains in ~1.5us.
"""

import ml_dtypes
import numpy as np
from contextlib import ExitStack

import concourse.bass as bass
import concourse.tile as tile
from concourse import mybir, bacc
from concourse import bass_utils
# Problem shape (hardcoded per contract)
B, CIN, COUT, H, W = 16, 128, 128, 128, 128
KH, KW = 3, 3
BLK = 32
NCORES = 8
BPC = B // NCORES            # images per core
PH, PW = H + 2, W + 2        # host zero-padded image (130 x 130)
FLAT = PH * PW

RPW = 4                      # output rows per PSUM window (N = 512 = full bank)
NWIN = H // RPW              # 32 windows
CHUNKS = [8, 8, 8, 4, 2, 2]  # windows per output-DMA chunk (tapered tail so the
                             # final y store drains fast; finer 1-window tails
                             # measured slower -- extra stores cost more in
                             # end-of-kernel completion waits than they save)
# One window-pair per matmul group: steady state runs at the 218ns/pair
# streaming limit with LDWEIGHTS hidden by the PE's pull-ahead, and the
# 8-bank PSUM pool holds 4 pairs in flight, so a pair's bank-reuse wait is
# always satisfied ~3 pairs ahead of time.  Larger groups measured ~0.9us
# of PSUM-recycle stall at every group boundary.
GROUPS = [[1] * c for c in CHUNKS]

_cache = {}
_last_in_maps = None


def _build(n_ib, paired):
    """Build + compile the per-core SPMD program.

    n_ib:   number of surviving 32-channel input blocks (1..4)
    paired: both images share the partition dim on separate PE row groups
    """
    K_used = BLK * n_ib
    reps = 2 if paired else 1
    DK = reps * K_used
    assert DK <= 128

    nc = bacc.Bacc("TRN2", target_bir_lowering=False, debug=False)
    f32 = mybir.dt.float32
    bf16 = mybir.dt.bfloat16

    # x and wT arrive in DRAM already bf16 (host converts): halves the x HBM
    # read and removes every on-chip cast from the load critical path.
    x_in = nc.dram_tensor("x", [BPC, K_used, PH, PW], bf16, kind="ExternalInput").ap()
    w_in = nc.dram_tensor("wt", [DK, KH * KW, COUT], bf16, kind="ExternalInput").ap()
    b_in = nc.dram_tensor("bias", [COUT], f32, kind="ExternalInput").ap()
    y_out = nc.dram_tensor("y", [BPC, COUT, H, W], f32, kind="ExternalOutput").ap()

    # x-load chunk boundaries (padded-image rows).  Window w reads padded rows
    # 4w..4w+5, so a boundary at 4k+6 releases windows 0..k.  Fine-grained
    # early chunks let the matmul pipeline start early; coarser later ones
    # keep the DMA count low.  (Merging to 8 chunks / splitting the weight
    # load / moving chunk 0 to the scalar ring all measured ~5us slower.)
    bounds = [0, 6, 14, 22, 30]
    while bounds[-1] + 16 < PH:
        bounds.append(bounds[-1] + 16)
    bounds.append(PH)

    with tile.TileContext(nc) as tc:
        with ExitStack() as ctx:
            singles = ctx.enter_context(tc.tile_pool(name="singles", bufs=1))
            stage_pool = ctx.enter_context(tc.tile_pool(name="ystage", bufs=4))
            psum_pool = ctx.enter_context(
                tc.tile_pool(name="psum", bufs=8, space="PSUM")
            )

            # Weights + bias on the Activation HWDGE ring so they drain in
            # parallel with the x chunks on the SP ring.  (Splitting the
            # tap-0 weight slice / moving chunk 0 to the scalar ring cuts
            # first-matmul time 13.3 -> 12.0us, but the gain is smaller than
            # the +-3us run-to-run P-state clock variance and never measured
            # a net win; kept simple instead.)
            wT = singles.tile([DK, KH * KW, COUT], bf16, name="wT2")
            nc.scalar.dma_start(out=wT, in_=w_in)
            bias_sb = singles.tile([COUT, 1], f32, name="bias_sb")
            nc.scalar.dma_start(out=bias_sb, in_=b_in.unsqueeze(1))

            def load_chunks(xbt, src):
                for k in range(len(bounds) - 1):
                    lo, hi = bounds[k] * PW, bounds[k + 1] * PW
                    nc.sync.dma_start(out=xbt[:, lo:hi], in_=src[:, lo:hi])

            if paired:
                xb = singles.tile([DK, FLAT], bf16, name="xb")
                load_chunks(xb, x_in.rearrange("b c h w -> (b c) (h w)"))
                xbufs = [xb] * BPC
                img_base = [i * K_used for i in range(BPC)]
            else:
                xbufs, img_base = [], []
                for b in range(BPC):
                    xbi = singles.tile([K_used, FLAT], bf16, name=f"xb{b}")
                    load_chunks(xbi, x_in[b].rearrange("c h w -> c (h w)"))
                    xbufs.append(xbi)
                    img_base.append(0)

            assert sum(CHUNKS) == NWIN
            assert [sum(g) for g in GROUPS] == CHUNKS
            c0 = 0
            for nwc, chunk_groups in zip(CHUNKS, GROUPS):
                wins = list(range(c0, c0 + nwc))
                chunk_r0 = RPW * c0
                chunk_nr = RPW * len(wins)
                c0 += nwc
                stages = [
                    stage_pool.tile(
                        [COUT, RPW * max(CHUNKS), W], f32, tag="stage",
                        name=f"st{b}_{chunk_r0}",
                    )
                    for b in range(BPC)
                ]
                g0 = 0
                for gsz in chunk_groups:
                    group = wins[g0 : g0 + gsz]
                    g0 += gsz
                    ps = {}
                    for b in range(BPC):
                        for w in group:
                            ps[(b, w)] = psum_pool.tile(
                                [128, 512], f32, tag="ps", name=f"ps{b}_{w}"
                            )
                    # tap-outer, image-inner: the two images' row groups
                    # alternate so each group's LDWEIGHTS hides behind the
                    # other group's matmuls.
                    for t in range(KH * KW):
                        dh, dw = divmod(t, KW)
                        for b in range(BPC):
                            base = img_base[b]
                            xbi = xbufs[b]
                            lhsT = wT[base : base + K_used, t, :]
                            for w in group:
                                q0 = (RPW * w + dh) * PW + dw
                                v = xbi[base : base + K_used, q0 : q0 + 1]
                                rhs = bass.AP(
                                    tensor=v.tensor,
                                    offset=v.offset,
                                    ap=[list(v.ap[0]), [PW, RPW], [1, W]],
                                )
                                nc.tensor.matmul(
                                    ps[(b, w)][:, : RPW * W],
                                    lhsT,
                                    rhs,
                                    start=(t == 0),
                                    stop=(t == KH * KW - 1),
                                )
                    # copy-out with fused bias, split across ACT and DVE
                    for b in range(BPC):
                        for w in group:
                            r0 = RPW * w
                            ps_v = ps[(b, w)][:, : RPW * W].rearrange(
                                "p (r s) -> p r s", s=W
                            )
                            dst = stages[b][:, r0 - chunk_r0 : r0 - chunk_r0 + RPW, :]
                            if (w + b) % 2 == 0:
                                nc.scalar.activation(
                                    out=dst,
                                    in_=ps_v,
                                    func=mybir.ActivationFunctionType.Identity,
                                    bias=bias_sb,
                                    scale=1.0,
                                )
                            else:
                                nc.vector.tensor_scalar_add(
                                    out=dst, in0=ps_v, scalar1=bias_sb
                                )
                # alternate the two HWDGE rings (SP / Activation) so the small
                # final stores don't queue behind every earlier large store
                for b in range(BPC):
                    eng = nc.sync if b == 0 else nc.scalar
                    eng.dma_start(
                        out=y_out[b][:, chunk_r0 : chunk_r0 + chunk_nr, :],
                        in_=stages[b][:, :chunk_nr, :],
                    )

    nc.compile()
    return nc


def kernel(x, weight, bias, mask):
    x = np.ascontiguousarray(np.asarray(x, dtype=np.float32))
    weight = np.asarray(weight, dtype=np.float32)
    bias = np.ascontiguousarray(np.asarray(bias, dtype=np.float32))
    mask = np.asarray(mask, dtype=np.float32)

    # --- host-side schedule specialization from the runtime mask ----------
    wm = weight * mask
    blk_any = (
        np.abs(wm).reshape(COUT, CIN // BLK, BLK, KH, KW).sum(axis=(0, 2, 3, 4)) > 0
    )
    used_ibs = [ib for ib in range(CIN // BLK) if blk_any[ib]] or [0]
    n_ib = len(used_ibs)
    K_used = BLK * n_ib
    paired = (BPC == 2) and (2 * K_used <= 128)
    reps = 2 if paired else 1

    used_ch = np.concatenate(
        [np.arange(ib * BLK, (ib + 1) * BLK) for ib in used_ibs]
    )

    key = (n_ib, paired)
    if key not in _cache:
        _cache[key] = _build(n_ib, paired)
    nc = _cache[key]

    # wT[rep*K_used + c, t, o] = (w*m)[o, used_ch[c], tap t], host-cast bf16
    wT = wm[:, used_ch].reshape(COUT, K_used, KH * KW).transpose(1, 2, 0)
    wT = np.ascontiguousarray(
        np.concatenate([wT] * reps, axis=0).astype(ml_dtypes.bfloat16)
    )

    # zero-padded x (130x130) restricted to the used channels, host-cast bf16
    xp = np.zeros((B, K_used, PH, PW), dtype=ml_dtypes.bfloat16)
    xp[:, :, 1 : H + 1, 1 : W + 1] = x[:, used_ch].astype(ml_dtypes.bfloat16)

    in_maps = []
    for core in range(NCORES):
        xs = np.ascontiguousarray(xp[core * BPC : (core + 1) * BPC])
        in_maps.append({"x": xs, "wt": wT, "bias": bias})

    global _last_in_maps
    _last_in_maps = in_maps

    res = bass_utils.run_bass_kernel_spmd(nc, in_maps, core_ids=list(range(NCORES)))
    y = np.concatenate([res.results[c]["y"] for c in range(NCORES)], axis=0)
    return y
